# revision 6
# baseline (speedup 1.0000x reference)
"""Bass/Trainium2 kernel for nn_KVCacheManager (untile + slice + stack KV cache).

Reference semantics:
  k_cache: (B, H, D, 128, T)  -> k = reshape(B,H,D,128*T)[..., :seq_len]   (BHDS)
  v_cache: (B, H, 128, T, D)  -> v = reshape(B,H,128*T,D)[:, :, :seq_len]  (BHSD)
  out = stack([swapaxes(k, 2, 3), v])  -> (2, B, H, seq_len, D)

Sharding: kv-head dimension (axis 1, H=8) across 8 NeuronCores, one head per
core.  Each core copies V (pure DRAM->DRAM DMA) and transposes K (D,S)->(S,D)
on-chip via TensorE transpose through PSUM.

The K transpose uses stride-J column sets (J = S/128): transpose #j reads
columns s = j + J*p' (p'=0..127) so SBUF partition p' accumulates output rows
[p'*J, (p'+1)*J) contiguously -> the store DMA is 128 partitions x J*512B
contiguous runs (max-efficiency descriptors), mirroring the load DMA.
"""

import numpy as np

import concourse.bacc as bacc
import concourse.bass as bass
import concourse.mybir as mybir
import concourse.tile as tile
from concourse.bass_utils import run_bass_kernel_spmd
from concourse.masks import make_identity

B, H, D, TILE = 4, 8, 128, 128
N_CORES = 8
F32 = mybir.dt.float32

_program_cache: dict = {}


def _build_program(seq_len: int) -> bass.Bass:
    """Per-core program: k_in [B,128,S] -> out[0] transposed; v_in flat -> out[1]."""
    S = seq_len
    J = S // TILE          # transposes per (b,); rows-per-partition of out
    S_main = J * TILE
    rem = S - S_main       # tail rows when seq_len % 128 != 0

    nc = bacc.Bacc("TRN2", target_bir_lowering=False, debug=False)
    k_in = nc.dram_tensor("k_in", [B, D, S], F32, kind="ExternalInput").ap()
    v_in = nc.dram_tensor("v_in", [B, S * D], F32, kind="ExternalInput").ap()
    out = nc.dram_tensor("out", [2, B, S, D], F32, kind="ExternalOutput").ap()

    with tile.TileContext(nc) as tc:
        with (
            tc.tile_pool(name="consts", bufs=1) as consts,
            tc.tile_pool(name="kin", bufs=2) as kin_pool,
            tc.tile_pool(name="kout", bufs=2) as kout_pool,
            tc.tile_pool(name="psum", bufs=6, space="PSUM") as psum_pool,
        ):
            ident = consts.tile([TILE, TILE], F32)
            make_identity(nc, ident)

            for b in range(B):
                # V: straight DRAM->DRAM copy, 3MB contiguous.
                nc.gpsimd.dma_start(out[1, b].rearrange("s d -> (s d)"), v_in[b])

                # K load: [128 d-partitions, S] with contiguous per-partition rows.
                kt = kin_pool.tile([D, S], F32)
                nc.sync.dma_start(kt[:], k_in[b])

                ot = kout_pool.tile([D, S], F32)
                if J > 0:
                    # view columns as (p', j): s = p'*J + j
                    ktv = kt[:].rearrange("d (p j) -> d p j", j=J)
                    # groups of 4 transposes share one PSUM bank [128, 512]
                    g4 = J // 4
                    for g in range(g4):
                        pt = psum_pool.tile([TILE, 4 * TILE], F32, tag="pt")
                        for u in range(4):
                            j = g * 4 + u
                            nc.tensor.transpose(
                                pt[:, u * TILE:(u + 1) * TILE], ktv[:, :, j], ident[:]
                            )
                        nc.vector.tensor_copy(
                            ot[:, g * 4 * TILE:(g + 1) * 4 * TILE], pt[:]
                        )
                    for j in range(g4 * 4, J):
                        pt1 = psum_pool.tile([TILE, TILE], F32, tag="pt1")
                        nc.tensor.transpose(pt1[:], ktv[:, :, j], ident[:])
                        nc.vector.tensor_copy(
                            ot[:, j * TILE:(j + 1) * TILE], pt1[:]
                        )
                    # store: partition p' holds rows [p'*J,(p'+1)*J) -> contiguous
                    nc.scalar.dma_start(
                        out[0, b, 0:S_main, :].rearrange("(p j) d -> p (j d)", p=D),
                        ot[:, 0:S_main],
                    )
                if rem:
                    ptr = psum_pool.tile([rem, TILE], F32, tag="ptr")
                    otr = kout_pool.tile([rem, TILE], F32, tag="otr")
                    nc.tensor.transpose(ptr[:], kt[:, S_main:S], ident[:])
                    nc.vector.tensor_copy(otr[:], ptr[:])
                    nc.scalar.dma_start(out[0, b, S_main:S, :], otr[:])

    nc.compile()
    return nc


def kernel(k_cache: np.ndarray, v_cache: np.ndarray, seq_len) -> np.ndarray:
    S = int(seq_len)
    k_cache = np.asarray(k_cache, dtype=np.float32)
    v_cache = np.asarray(v_cache, dtype=np.float32)
    assert k_cache.shape == (B, H, D, TILE, k_cache.shape[4])
    T = k_cache.shape[4]

    if S == 0:
        return np.zeros((2, B, H, 0, D), dtype=np.float32)

    # Host-side shard prep: slice seq to S, one head per core.
    k_flat = k_cache.reshape(B, H, D, TILE * T)[:, :, :, :S]        # (B,H,D,S)
    v_flat = v_cache.reshape(B, H, TILE * T, D)[:, :, :S, :]        # (B,H,S,D)

    in_maps = []
    for h in range(N_CORES):
        in_maps.append({
            "k_in": np.ascontiguousarray(k_flat[:, h]),              # (B,D,S)
            "v_in": np.ascontiguousarray(v_flat[:, h]).reshape(B, S * D),
        })

    key = S
    if key not in _program_cache:
        _program_cache[key] = _build_program(S)
    nc = _program_cache[key]

    results = run_bass_kernel_spmd(nc, in_maps, core_ids=list(range(N_CORES)))

    out = np.empty((2, B, H, S, D), dtype=np.float32)
    for h in range(N_CORES):
        out[:, :, h] = results.results[h]["out"]
    return out
